# revision 45
# baseline (speedup 1.0000x reference)
"""MLA (multi-head latent attention) Trainium2 kernel, 8-core SPMD.

Sharding: core c handles batch b = c//4 and heads 4*(c%4) .. 4*(c%4)+4.
The small compression projections (Wd, Wqd) are replicated; up/rope/Wo are
head-sharded. Each core returns a partial [S, D] output (its heads' slice of
the row-sharded Wo matmul); the host sums the 4 partials per batch and adds bo.

Per-core pipeline (fp32 storage, fp32r matmuls on the PE):
  P1: kv_cT = (x @ Wd + bd)^T and kT_r = rope(x @ Wkr + bkr)^T
  P2: kv_upT = (kv_c @ Wu + bu)^T   (serves as both K-content^T and V^T)
  P3: q_cmpT = (x @ Wqd + bqd)^T
  P4: qT_c, qT_r (transposed q branches)
  P5: per head: scoresT[k,q] -> exp -> probsT; out_T[dh,q] = V^T @ probsT;
      softmax denominators via in-place tree-add + ones-matmul (no max
      subtraction needed: |scores*scale| < ~1.5); normalize at evacuation.
  P6: partial = attn_flat @ Wo_heads  (bo added on host after the reduce).
"""

import sys
import types

import numpy as np

import concourse.bass as bass
import concourse.tile as tile
from concourse import mybir, bacc
from concourse.bass_utils import run_bass_kernel_spmd
from concourse.masks import make_identity

try:  # degrade gracefully if BASS_TRACE is set but the axon NTFF hook is absent
    import antenv.axon_hooks  # noqa: F401
except ImportError:
    _m = types.ModuleType("antenv.axon_hooks")
    _m.get_axon_ntff_profile_hook = lambda: None
    sys.modules["antenv.axon_hooks"] = _m

F32 = mybir.dt.float32
F32R = mybir.dt.float32r
AF = mybir.ActivationFunctionType

B, S, D = 2, 2048, 2048
H, DH, DR = 16, 128, 64
DC, DQ = 512, 768
HPC = 4              # heads per core
NCORES = 8
P = 128
ND = D // P          # 16
NDC = DC // P        # 4
NDQ = DQ // P        # 6
NS = S // P          # 16 (128-wide s chunks)
SC = S // 512        # 4  (512-wide s chunks)
KCH = S // P         # 16 key chunks
QBLK = 512
NQB = S // QBLK      # 4
SCALE = float(1.0 / np.sqrt(np.float32(DH)))
ROPE_THETA = 10000.0

_NC_CACHE = {}


class _Pools:
    """Tile pools with explicit lifetimes (LIFO per (space, side) stack)."""

    def __init__(self, tc):
        self.tc = tc
        self._cms = {}
        self._order = []

    def enter(self, name, **kw):
        cm = self.tc.tile_pool(name=name, **kw)
        pool = cm.__enter__()
        self._cms[name] = cm
        self._order.append(name)
        return pool

    def exit(self, *names):
        for name in sorted(names, key=self._order.index, reverse=True):
            self._cms.pop(name).__exit__(None, None, None)
            self._order.remove(name)

    def exit_all(self):
        self.exit(*list(self._cms))


def _bcast_ap(t, n):
    """DRAM [n] vector -> AP replicated over P partitions."""
    ap = t.ap()
    return bass.AP(tensor=ap.tensor, offset=ap.offset, ap=[[0, P], [1, n]])


def _emit_rope(nc, pool, prps, out_t, bias_b, cos_ap, sin_ap):
    """prps: psum [P, HPC, DR] (pre-rope proj), out_t: sbuf [P, HPC, DR] f32.

    Rope pairs are host-permuted to deinterleaved layout: per head the first
    32 dims are x1 (even original dims), last 32 are x2 (odd)."""
    pre = pool.tile([P, HPC, DR], F32, tag="rope_pre")
    nc.any.tensor_add(pre[:], prps[:], bias_b[:])
    x1 = pre[:, :, 0:32]
    x2 = pre[:, :, 32:64]
    c = cos_ap[:, None, :].to_broadcast((P, HPC, 32))
    s = sin_ap[:, None, :].to_broadcast((P, HPC, 32))
    t1 = pool.tile([P, HPC, 32], F32, tag="rope_t1")
    t2 = pool.tile([P, HPC, 32], F32, tag="rope_t2")
    nc.any.tensor_mul(t1[:], x1, c)
    nc.any.tensor_mul(t2[:], x2, s)
    nc.any.tensor_sub(out_t[:, :, 0:32], t1[:], t2[:])
    t3 = pool.tile([P, HPC, 32], F32, tag="rope_t3")
    t4 = pool.tile([P, HPC, 32], F32, tag="rope_t4")
    nc.any.tensor_mul(t3[:], x1, s)
    nc.any.tensor_mul(t4[:], x2, c)
    nc.any.tensor_add(out_t[:, :, 32:64], t3[:], t4[:])


def _build_nc():
    nc = bacc.Bacc("TRN2", target_bir_lowering=False, debug=False)

    # x^T arrives pre-tiled: [s-block, p, o, s-in-block] (256-wide blocks)
    xT = nc.dram_tensor("xT", [S // 256, P, ND, 256], F32R, kind="ExternalInput")
    # weights arrive pre-tiled to partition-major [P, chunks*cols] layout
    Wd = nc.dram_tensor("Wd", [P, ND * DC], F32R, kind="ExternalInput")
    Wqd = nc.dram_tensor("Wqd", [P, ND * DQ], F32R, kind="ExternalInput")
    Wkr = nc.dram_tensor("Wkr", [P, ND * HPC * DR], F32R, kind="ExternalInput")
    Wu = nc.dram_tensor("Wu", [P, NDC * HPC * DH], F32R, kind="ExternalInput")
    Wqu = nc.dram_tensor("Wqu", [P, NDQ * HPC * DH], F32R, kind="ExternalInput")
    Wqr = nc.dram_tensor("Wqr", [P, NDQ * HPC * DR], F32R, kind="ExternalInput")
    Wo = nc.dram_tensor("Wo", [P, HPC * D], F32R, kind="ExternalInput")
    bd = nc.dram_tensor("bd", [DC], F32, kind="ExternalInput")
    bqd = nc.dram_tensor("bqd", [DQ], F32, kind="ExternalInput")
    bu = nc.dram_tensor("bu", [HPC * DH], F32, kind="ExternalInput")
    bqu = nc.dram_tensor("bqu", [HPC * DH], F32, kind="ExternalInput")
    bqr = nc.dram_tensor("bqr", [HPC * DR], F32, kind="ExternalInput")
    bkr = nc.dram_tensor("bkr", [HPC * DR], F32, kind="ExternalInput")
    cosn = nc.dram_tensor("cosn", [S, DR // 2], F32, kind="ExternalInput")
    sinn = nc.dram_tensor("sinn", [S, DR // 2], F32, kind="ExternalInput")
    partial = nc.dram_tensor("partial", [S, D], F32, kind="ExternalOutput")

    xT_b = xT.ap()
    wqd_v = Wqd.ap().rearrange("p (o c) -> p o c", o=ND)
    wd_v = Wd.ap().rearrange("p (o c) -> p o c", o=ND)
    out_v = partial.ap().rearrange("(o p) n -> p o n", p=P)

    with tile.TileContext(nc) as tc:
        pl = _Pools(tc)
        misc = pl.enter("misc", bufs=1)
        krp = pl.enter("krp", bufs=1)

        p6ps = pl.enter("p6ps", bufs=2, space="PSUM")

        ident = misc.tile([P, P], F32)
        make_identity(nc, ident)
        identr_t = misc.tile([P, P], F32R)
        nc.vector.tensor_copy(identr_t[:], ident[:])
        identr = identr_t[:]
        ones_f = misc.tile([P, 1], F32)
        nc.vector.memset(ones_f, 1.0)
        ones_s = misc.tile([P, 1], F32R)
        nc.vector.tensor_copy(ones_s[:], ones_f[:])

        kT_r = krp.tile([P, 2, S], F32R)       # head h: parts (h%2)*64.., idx h//2

        bqd_s = misc.tile([P, NDQ], F32)
        bqu_s = misc.tile([P, HPC], F32)
        bqr_b = misc.tile([P, HPC, DR], F32)

        tblc = pl.enter("tblc", bufs=1)        # cos/sin, P1..P4
        cos_s = tblc.tile([P, NS, DR // 2], F32)
        sin_s = tblc.tile([P, NS, DR // 2], F32)

        # ---------------- P1: kv_cT + kT_r ----------------
        tkv = pl.enter("tkv", bufs=1)          # kv-side biases, P1..P2
        bd_s = tkv.tile([P, NDC], F32)
        bu_s = tkv.tile([P, HPC], F32)
        bkr_b = tkv.tile([P, HPC, DR], F32)

        kvcp = pl.enter("kvcp", bufs=1)
        kv_cT = kvcp.tile([P, NDC, S], F32R)

        p2w = pl.enter("p2w", bufs=1)
        p1w = pl.enter("p1w", bufs=1)
        lp = pl.enter("p1loc", bufs=2, side="right")
        lp1 = pl.enter("p1st", bufs=1)
        ps1 = pl.enter("p1ps", bufs=2, space="PSUM")
        ps2 = pl.enter("p1ps2", bufs=2, space="PSUM")
        pst = pl.enter("p1pst", bufs=2, space="PSUM")

        # startup order: first two contiguous 256-wide x blocks, then weights
        xch0a = lp1.tile([P, ND, 256], F32R, tag="xsta")
        wd_s = p1w.tile([P, ND, DC], F32R)
        # finely pieced first loads so the first matmuls start ASAP
        nc.sync.dma_start(xch0a[:, 0:4, :], xT_b[0, :, 0:4, :])
        nc.gpsimd.dma_start(wd_s[:, 0:2, :], wd_v[:, 0:2, :])
        nc.gpsimd.dma_start(wd_s[:, 2:4, :], wd_v[:, 2:4, :])
        for og in range(1, 4):
            nc.sync.dma_start(
                xch0a[:, 4 * og:4 * og + 4, :], xT_b[0, :, 4 * og:4 * og + 4, :])
        for og in range(1, 4):
            nc.gpsimd.dma_start(
                wd_s[:, 4 * og:4 * og + 4, :], wd_v[:, 4 * og:4 * og + 4, :])

        wkr_s = p1w.tile([P, ND, HPC * DR], F32R)
        nc.gpsimd.dma_start(
            wkr_s[:], Wkr.ap().rearrange("p (o c) -> p o c", o=ND))
        wu_s = p2w.tile([P, NDC, HPC * DH], F32R)
        nc.sync.dma_start(bd_s[:], bd.ap().rearrange("(o p) -> p o", p=P))
        nc.sync.dma_start(cos_s[:], cosn.ap().rearrange("(o p) i -> p o i", p=P))
        nc.sync.dma_start(sin_s[:], sinn.ap().rearrange("(o p) i -> p o i", p=P))
        nc.sync.dma_start(bu_s[:], bu.ap().rearrange("(o p) -> p o", p=P))
        nc.gpsimd.dma_start(bkr_b[:], _bcast_ap(bkr, HPC * DR))

        # first and last chunks are 256-wide (fast start / budget), rest 512
        chunks = [(0, 256, xch0a)] + [
            (o, 512, None) for o in range(256, S - 256, 512)] + [
            (S - 256, 256, "last")]
        for ci, (off, width, xch) in enumerate(chunks):
            coff = 0
            blk = off // 256
            if xch == "last":
                xch = lp1.tile([P, ND, 256], F32R, tag="xsta", name="xchl")
                nc.sync.dma_start(xch[:], xT_b[blk])
            elif xch is None:
                xch = lp.tile([P, ND, 512], F32R, tag="xch")
                nc.sync.dma_start(xch[:, :, 0:256], xT_b[blk])
                nc.sync.dma_start(xch[:, :, 256:512], xT_b[blk + 1])
            if ci == 1:
                # prefetch Wu during P1 (after the startup-critical loads)
                nc.gpsimd.dma_start(
                    wu_s[:], Wu.ap().rearrange("p (o c) -> p o c", o=NDC))
            if ci == 2:
                nc.sync.dma_start(
                    bqd_s[:], bqd.ap().rearrange("(o p) -> p o", p=P))
                nc.sync.dma_start(
                    bqu_s[:], bqu.ap().rearrange("(o p) -> p o", p=P))
                nc.gpsimd.dma_start(bqr_b[:], _bcast_ap(bqr, HPC * DR))
            for cc in range(NDC):
                psum = ps1.tile([P, 512], F32, name="psum")[:, :width]
                for kc in range(ND):
                    nc.tensor.matmul(
                        psum[:], wd_s[:, kc, cc * P:(cc + 1) * P],
                        xch[:, kc, coff:coff + width],
                        start=(kc == 0), stop=(kc == ND - 1))
                nc.any.tensor_scalar_add(
                    kv_cT[:, cc, off:off + width],
                    psum[:], bd_s[:, cc:cc + 1])
            for s2 in range(width // P):
                ssc = (off // P) + s2
                prps = ps2.tile([P, HPC, DR], F32)
                for kc in range(ND):
                    nc.tensor.matmul(
                        prps[:],
                        xch[:, kc, coff + s2 * P:coff + (s2 + 1) * P],
                        wkr_s[:, kc, :],
                        start=(kc == 0), stop=(kc == ND - 1))
                krn = lp.tile([P, HPC, DR], F32R, tag="krn")
                _emit_rope(nc, lp, prps, krn, bkr_b,
                           cos_s[:, ssc, :], sin_s[:, ssc, :])
                for j in range(2):
                    tp = pst.tile([P, P], F32R)
                    nc.tensor.transpose(
                        tp[:], krn[:, 2 * j:2 * j + 2, :], identr)
                    nc.any.tensor_copy(
                        kT_r[:, j, ssc * P:(ssc + 1) * P], tp[:])

        pl.exit("p1loc", "p1st", "p1w", "p1ps", "p1ps2", "p1pst")

        # ---------------- P2: kv_upT ----------------
        kvupp = pl.enter("kvupp", bufs=1, side="right")
        kvupT = kvupp.tile([P, HPC, S], F32R)
        p2ps = pl.enter("p2ps", bufs=3, space="PSUM")
        for sc in range(SC):
            for h in range(HPC):
                psum = p2ps.tile([P, 512], F32)
                for kc in range(NDC):
                    nc.tensor.matmul(
                        psum[:],
                        wu_s[:, kc, h * DH:(h + 1) * DH],
                        kv_cT[:, kc, sc * 512:(sc + 1) * 512],
                        start=(kc == 0), stop=(kc == NDC - 1))
                nc.any.tensor_scalar_add(
                    kvupT[:, h, sc * 512:(sc + 1) * 512],
                    psum[:], bu_s[:, h:h + 1])
        pl.exit("p2w", "p2ps", "kvcp", "tkv")

        # ---------------- P3: q_cmpT ----------------
        qcp = pl.enter("qcp", bufs=1)
        q_cmpT = qcp.tile([P, NDQ, S], F32R)
        p4w = pl.enter("p4w", bufs=1)
        wqu_s = p4w.tile([P, NDQ, HPC * DH], F32R)
        nc.gpsimd.dma_start(wqu_s[:], Wqu.ap().rearrange("p (o c) -> p o c", o=NDQ))
        wqr_s = p4w.tile([P, NDQ, HPC * DR], F32R)
        nc.gpsimd.dma_start(wqr_s[:], Wqr.ap().rearrange("p (o c) -> p o c", o=NDQ))
        p3w = pl.enter("p3w", bufs=1, side="right")
        wqd_s = p3w.tile([P, ND, DQ], F32R)
        for og in range(4):
            nc.gpsimd.dma_start(
                wqd_s[:, 4 * og:4 * og + 4, :], wqd_v[:, 4 * og:4 * og + 4, :])
        lp = pl.enter("p3loc", bufs=2, side="right")
        p3ps = pl.enter("p3ps", bufs=4, space="PSUM")
        NXCH = S // 256
        for xc in range(NXCH):
            xch = lp.tile([P, ND, 256], F32R, tag="xch3")
            nc.sync.dma_start(xch[:], xT_b[xc])
            for cc in range(NDQ):
                if xc == 0 and cc < 2:
                    # borrow the (still idle) reserved P6 psum pool so the
                    # first groups don't wait for P1's psum release
                    psum = p6ps.tile([P, 512], F32, tag="psum", name="p3boot")[:, :256]
                else:
                    psum = p3ps.tile([P, 256], F32)
                for kc in range(ND):
                    nc.tensor.matmul(
                        psum[:], wqd_s[:, kc, cc * P:(cc + 1) * P], xch[:, kc, :],
                        start=(kc == 0), stop=(kc == ND - 1))
                nc.any.tensor_scalar_add(
                    q_cmpT[:, cc, xc * 256:(xc + 1) * 256],
                    psum[:], bqd_s[:, cc:cc + 1])
        pl.exit("p3loc", "p3ps", "p3w")

        # ---------------- P4: qT_c + qT_r ----------------
        qp = pl.enter("qp", bufs=1, side="right")
        qT_c = qp.tile([P, HPC, S], F32R)
        qT_r = qp.tile([P, 2, S], F32R)
        lp = pl.enter("p4loc", bufs=2)
        p4ps = pl.enter("p4ps", bufs=2, space="PSUM")
        p4ps2 = pl.enter("p4ps2", bufs=2, space="PSUM")
        p4pst = pl.enter("p4pst", bufs=2, space="PSUM")
        # interleave the DVE-latency-bound rope pipeline with the PE-dense
        # qT_c matmuls so the PE never waits on the rope chain
        def emit_qtc(sc, h):
            psum = p4ps.tile([P, 512], F32)
            for kc in range(NDQ):
                nc.tensor.matmul(
                    psum[:],
                    wqu_s[:, kc, h * DH:(h + 1) * DH],
                    q_cmpT[:, kc, sc * 512:(sc + 1) * 512],
                    start=(kc == 0), stop=(kc == NDQ - 1))
            nc.any.tensor_scalar_add(
                qT_c[:, h, sc * 512:(sc + 1) * 512],
                psum[:], bqu_s[:, h:h + 1])

        for ssc in range(NS):
            prps = p4ps2.tile([P, HPC, DR], F32)
            for kc in range(NDQ):
                nc.tensor.matmul(
                    prps[:], q_cmpT[:, kc, ssc * P:(ssc + 1) * P], wqr_s[:, kc, :],
                    start=(kc == 0), stop=(kc == NDQ - 1))
            qrn = lp.tile([P, HPC, DR], F32R, tag="qrn")
            _emit_rope(nc, lp, prps, qrn, bqr_b,
                       cos_s[:, ssc, :], sin_s[:, ssc, :])
            for j in range(2):
                tp = p4pst.tile([P, P], F32R)
                nc.tensor.transpose(
                    tp[:], qrn[:, 2 * j:2 * j + 2, :], identr)
                nc.any.tensor_copy(qT_r[:, j, ssc * P:(ssc + 1) * P], tp[:])
            emit_qtc(ssc // 4, ssc % 4)
        pl.exit("p4loc", "p4w", "p4ps", "p4ps2", "p4pst", "qcp", "tblc")

        # ---------------- P5: attention ----------------
        p6w = pl.enter("p6w", bufs=2, side="right")
        wo_v = Wo.ap().rearrange("p (o n) -> p o n", o=HPC)
        wo_sls = []
        for ncc in range(4):
            wo_sl = p6w.tile([P, HPC, 512], F32R, tag="wo")
            nc.sync.dma_start(wo_sl[:], wo_v[:, :, ncc * 512:(ncc + 1) * 512])
            wo_sls.append(wo_sl)
        outp = pl.enter("outp", bufs=1)
        ap_ = pl.enter("attn", bufs=2)
        kvn_p = pl.enter("kvn", bufs=2)
        invp = pl.enter("invp", bufs=1)
        scps = pl.enter("scps", bufs=3, space="PSUM")
        avps = pl.enter("avps", bufs=2, space="PSUM")
        pst5 = pl.enter("p5pst", bufs=1, space="PSUM")

        outT = outp.tile([P, HPC, S], F32R)    # attention out^T, per head

        kvupn_tiles = []
        for _hh in range(HPC):
            kvupn_t = kvn_p.tile([P, KCH, DH], F32R, tag="kvupn", name=f"kvupn{_hh}")
            kvupn_tiles.append(kvupn_t)

        def emit_kvupn(hh, kc):
            tp = pst5.tile([P, P], F32R)
            nc.tensor.transpose(
                tp[:], kvupT[:, hh, kc * P:(kc + 1) * P], identr)
            nc.scalar.copy(kvupn_tiles[hh][:, kc, :], tp[:])

        for h in range(HPC):
            hb, hj = (h % 2) * 64, h // 2
            kvupn = kvupn_tiles[h]
            for qb in range(NQB):
                q0 = qb * QBLK
                # probsT in two 8-chunk halves: halves the SBUF footprint while
                # keeping cross-qb pipelining (slot of half A frees mid-block)
                pA = ap_.tile([P, KCH // 2, QBLK], F32R, tag="probsT")
                pB = ap_.tile([P, KCH // 2, QBLK], F32R, tag="probsT")
                halves = (pA, pB)
                av = avps.tile([P, QBLK], F32, tag="av", name="av")
                for kc in range(KCH):
                    ph, ki = halves[kc // 8], kc % 8
                    if h == 0 and qb == 0:
                        # head 0's V (kv_up normal layout) just ahead of use
                        emit_kvupn(0, kc)
                    if qb >= NQB - 2 and h + 1 < HPC:
                        # prefetch the next head's V spread over the tail qbs
                        half = qb - (NQB - 2)
                        if kc % 2 == half:
                            emit_kvupn(h + 1, 8 * (kc % 2) + kc // 2)
                    sps = scps.tile([P, QBLK], F32)
                    nc.tensor.matmul(
                        sps[:],
                        kvupT[:, h, kc * P:(kc + 1) * P],
                        qT_c[:, h, q0:q0 + QBLK],
                        start=True, stop=False)
                    nc.tensor.matmul(
                        sps[:],
                        kT_r[hb:hb + 64, hj, kc * P:(kc + 1) * P],
                        qT_r[hb:hb + 64, hj, q0:q0 + QBLK],
                        start=False, stop=True)
                    nc.scalar.activation(
                        ph[:, ki, :], sps[:], AF.Exp, scale=SCALE)
                    # AV accumulation interleaved per k-chunk keeps PE fed
                    # while ACT exps the next chunk.
                    nc.tensor.matmul(
                        av[:], kvupn[:, kc, :], ph[:, ki, :],
                        start=(kc == 0), stop=(kc == KCH - 1))
                    if kc == 7 or kc == KCH - 1:
                        # in-place tree reduction of the finished half
                        nc.any.tensor_add(
                            ph[:, 0:4, :], ph[:, 0:4, :], ph[:, 4:8, :])
                        nc.any.tensor_add(
                            ph[:, 0:2, :], ph[:, 0:2, :], ph[:, 2:4, :])
                        nc.any.tensor_add(
                            ph[:, 0:1, :], ph[:, 0:1, :], ph[:, 1:2, :])
                smp_t = avps.tile([P, QBLK], F32, tag="av", name="smp_t")
                smps = smp_t[0:1, :]
                nc.tensor.matmul(
                    smps, ones_s[:], pA[:, 0, :], start=True, stop=False)
                nc.tensor.matmul(
                    smps, ones_s[:], pB[:, 0, :], start=False, stop=True)
                inv = invp.tile([1, QBLK], F32, tag="inv")
                nc.vector.reciprocal(inv[:], smps)
                invb = invp.tile([P, QBLK], F32, tag="invb")
                nc.gpsimd.partition_broadcast(invb[:], inv[:])
                nc.any.tensor_mul(outT[:, h, q0:q0 + QBLK], av[:], invb[:])

        pl.exit("attn", "kvn", "invp", "scps", "avps", "p5pst")

        # ---------------- P6: output projection ----------------
        lp = pl.enter("p6loc", bufs=3, side="right")
        for ncc in range(4):
            wo_sl = wo_sls[ncc]
            for s16 in range(NS):
                psum = p6ps.tile([P, 512], F32)
                for h in range(HPC):
                    nc.tensor.matmul(
                        psum[:],
                        outT[:, h, s16 * P:(s16 + 1) * P],
                        wo_sl[:, h, :],
                        start=(h == 0), stop=(h == HPC - 1))
                osb = lp.tile([P, 512], F32, tag="osb")
                nc.any.tensor_copy(osb[:], psum[:])
                nc.gpsimd.dma_start(
                    out_v[:, s16, ncc * 512:(ncc + 1) * 512], osb[:])
        pl.exit_all()

    nc.compile()
    return nc


def _get_nc():
    if "nc" not in _NC_CACHE:
        _NC_CACHE["nc"] = _build_nc()
    return _NC_CACHE["nc"]


def _rope_tables():
    inv_freq = (1.0 / (ROPE_THETA ** (np.arange(0, DR, 2, dtype=np.float32) / DR)))
    t = np.arange(S, dtype=np.float32)
    ang = t[:, None] * inv_freq[None, :]
    return np.cos(ang).astype(np.float32), np.sin(ang).astype(np.float32)


def _pt(W):
    """[R, C] weight -> partition-major pre-tiled [128, (R//128)*C]."""
    R, C = W.shape
    return np.ascontiguousarray(
        W.reshape(R // P, P, C).transpose(1, 0, 2).reshape(P, -1))


def _shard_inputs(x, Wd, bd, Wu, bu, Wqd, bqd, Wqu, bqu, Wqr, bqr, Wkr, bkr, Wo):
    cosn, sinn = _rope_tables()
    perm = np.concatenate([np.arange(0, DR, 2), np.arange(1, DR, 2)])

    Wqr_h = Wqr.reshape(DQ, H, DR)[:, :, perm]
    Wkr_h = Wkr.reshape(D, H, DR)[:, :, perm]
    bqr_h = bqr.reshape(H, DR)[:, perm]
    bkr_h = bkr.reshape(H, DR)[:, perm]
    Wu_h = Wu.reshape(DC, H, DH)
    bu_h = bu.reshape(H, DH)
    Wqu_h = Wqu.reshape(DQ, H, DH)
    bqu_h = bqu.reshape(H, DH)
    Wo_h = Wo.reshape(H, DH, D)

    xT_t = [np.ascontiguousarray(
        x[b].T.reshape(ND, P, S // 256, 256).transpose(2, 1, 0, 3))
        for b in range(B)]
    in_maps = []
    for c in range(NCORES):
        b = c // 4
        hs = slice((c % 4) * HPC, (c % 4) * HPC + HPC)
        in_maps.append({
            "xT": xT_t[b],
            "Wd": _pt(Wd),
            "Wqd": _pt(Wqd),
            "Wkr": _pt(Wkr_h[:, hs].reshape(D, HPC * DR)),
            "Wu": _pt(Wu_h[:, hs].reshape(DC, HPC * DH)),
            "Wqu": _pt(Wqu_h[:, hs].reshape(DQ, HPC * DH)),
            "Wqr": _pt(Wqr_h[:, hs].reshape(DQ, HPC * DR)),
            "Wo": _pt(Wo_h[hs].reshape(HPC * DH, D)),
            "bd": bd,
            "bqd": bqd,
            "bu": np.ascontiguousarray(bu_h[hs].reshape(-1)),
            "bqu": np.ascontiguousarray(bqu_h[hs].reshape(-1)),
            "bqr": np.ascontiguousarray(bqr_h[hs].reshape(-1)),
            "bkr": np.ascontiguousarray(bkr_h[hs].reshape(-1)),
            "cosn": cosn,
            "sinn": sinn,
        })
    return in_maps


def kernel(x, Wd, bd, Wu, bu, Wqd, bqd, Wqu, bqu, Wqr, bqr, Wkr, bkr, Wo, bo):
    args = [np.ascontiguousarray(np.asarray(a, np.float32)) for a in
            (x, Wd, bd, Wu, bu, Wqd, bqd, Wqu, bqu, Wqr, bqr, Wkr, bkr, Wo)]
    bo = np.asarray(bo, np.float32)

    nc = _get_nc()
    in_maps = _shard_inputs(*args)
    res = run_bass_kernel_spmd(nc, in_maps, core_ids=list(range(NCORES)))

    out = np.zeros((B, S, D), np.float32)
    for c in range(NCORES):
        out[c // 4] += res.results[c]["partial"]
    out += bo[None, None, :]
    return out
